# revision 25
# baseline (speedup 1.0000x reference)
"""Trainium2 Bass kernel for nn_GameboyNet (sparse windowed attention net).

Sharding: pure data-parallel over batch — B=8 rows, one per NeuronCore.
Each core runs the full 32-layer network on its own (S=4096, D=256)
sequence, residual stream resident in SBUF feature-major (D x S) f32.

Key trick — centered fp8: the residual stream h is ~99% a data-independent
constant hbar (accumulated biases; computed on the host by running the
layer recurrence on the batch-mean embedding). The device quantizes only
dev = (h - hbar)*64 to fp8-e4m3 and all dense projections (QKV, W1, W2,
final conv) run as fp8 DoubleRow matmuls (2 MACs/cell/cycle, K=256 pairs);
the exact hbar contributions travel through f32 bias paths folded on the
host. The MLP sigmoid is likewise centered: u = 0.5 + 0.5*tanh(z/2), the
0.5*sum(W2) part folded into the bias, so fp8 only carries tanh deviations.
Attention itself (scores, exp, AV) stays bf16.

Attention (window W=512, causal, look_backward=1) is computed block-sparse
in transposed form: scoresT[k, q] = kT.T @ qT per 128-token key block, so
the AV matmul out[d, q] lands feature-major, matching the residual layout.
Softmax skips max-subtraction (scores are small for this data regime;
validated vs reference). Denominators: exp tiles tree-added on DVE, one
ones[128x128] matmul per window broadcasts column sums to all partitions,
reciprocal_approx_fast gives 1/sum; normalization multiplies the AV psum
directly (software-pipelined one window behind). The v bias and Wv@hbar
pass through softmax exactly (weights sum to 1) and are folded into b1/b2.
"""
import os
import sys
import types

sys.path.insert(0, '/opt/trn_rl_repo')

import numpy as np
import ml_dtypes

import concourse.bass as bass
import concourse.mybir as mybir
import concourse.tile as tile
from concourse import bacc
from concourse.bass import ds
from concourse.bass_utils import run_bass_kernel_spmd

B, S, D, W, L = 8, 4096, 256, 512, 32
E = 4 * D
NW = S // W
P = 128
DC = D // P          # 2 d-chunks
EC = E // P          # 8 e-chunks
TT = S // 512        # 8 token tiles of 512
TB = S // P          # 32 token blocks of 128
BN_EPS = 1e-5
NEG = -1e9

DEV_S = 64.0         # dev = (h - hbar) * 2^6
WT_S = 16.0          # fp8 weights * 2^4
W2_S = 32.0          # w2' = 0.5*W2 * 2^5
QK_INV = 1.0 / 16.0  # 1/sqrt(D)
SC_QK8 = 1.0 / 32.0    # q/k psum -> fp8 qT/kT at 32*{q,k}_dev
SC_EXP = QK_INV / (DEV_S * WT_S)       # exp scale: scores psum * 2^-14
SC_V = 1.0 / WT_S                      # vtm8 = v_dev * DEV_S (fp8 range)
SC_U = 0.5 / (DEV_S * WT_S)            # tanh(z/2): z = psum*2^-10
SC_F = 1.0 / (DEV_S * WT_S)

f32 = mybir.dt.float32
bf16 = mybir.dt.bfloat16
f8 = mybir.dt.float8e4
f8np = ml_dtypes.float8_e4m3
AF = mybir.ActivationFunctionType
ALU = mybir.AluOpType
DR = mybir.MatmulPerfMode.DoubleRow

LAST_EXEC_NS = None
LAST_TRACE = None

_cache = {}


def _install_ntff_hook():
    """The agent image's antenv is a stub without axon_hooks; inject it so
    trace=True can capture NTFF profiles through the axon tunnel."""
    try:
        import antenv
        if 'antenv.axon_hooks' in sys.modules:
            return
        mod = types.ModuleType("antenv.axon_hooks")
        _HOOK = [None]
        mod.set_axon_ntff_profile_hook = lambda h: _HOOK.__setitem__(0, h)
        mod.get_axon_ntff_profile_hook = lambda: _HOOK[0]
        sys.modules["antenv.axon_hooks"] = mod
        antenv.axon_hooks = mod
        from trn_agent_boot.trn_boot import _ntff_profile_via_ctypes
        hook = _ntff_profile_via_ctypes('/opt/axon/libaxon_pjrt.so')
        mod.set_axon_ntff_profile_hook(hook)
    except Exception:
        pass


def _emit_layer(nc, tc, pools, loff):
    """Emit one transformer layer. loff = layer index (int or RV)."""
    (wpool, psum, psum2, expp, rbp, tmpp, usb,
     hT, dev8, qT, kT, vtm, ones8, maskb, idb) = pools

    dma = nc.sync.dma_start

    # ---- per-layer weight loads --------------------------------------
    wq_sb = wpool.tile([P, DC, D], f8, tag="wq")
    wk_sb = wpool.tile([P, DC, D], f8, tag="wk")
    wv_sb = wpool.tile([P, DC, D], f8, tag="wv")
    w1_sb = wpool.tile([P, DC, E], f8, tag="w1")
    w2_sb = wpool.tile([P, EC // 2, 2, D], f8, tag="w2")
    cons = wpool.tile([P, 22], f32, tag="cons")
    qbb = wpool.tile([P, DC], bf16, tag="qbb")

    dma(out=wq_sb[:, :, :], in_=nc.t_wq8[ds(loff * P, P), :])
    dma(out=wk_sb[:, :, :], in_=nc.t_wk8[ds(loff * P, P), :])
    dma(out=wv_sb[:, :, :], in_=nc.t_wv8[ds(loff * P, P), :])
    dma(out=w1_sb[:, :, :], in_=nc.t_w18[ds(loff * P, P), :])
    dma(out=w2_sb[:, :, :, :], in_=nc.t_w28[ds(loff * P, P), :])
    dma(out=cons, in_=nc.t_cons[ds(loff * P, P), :])
    dma(out=qbb, in_=nc.t_qbb[ds(loff * P, P), :])
    # cons cols: 0:2 qbias/16, 2:4 kbias, 4:12 b1t, 12:14 A, 14:16 Cfull,
    #            16:18 A*2^-5, 18:20 hbar, 20:22 -64*hbar

    # ---- QKV interleaved with attention: window w's matmuls run right
    # after token tile w's projections, so the QKV-phase DVE drains overlap
    # the attention-phase PE work instead of throttling it.
    # The q/k biases are dropped: only the per-key-token term (qbias . k_dev)
    # survives softmax, computed as tiny N=1 matmuls and applied as exp bias.
    ps_t = psum2.tile([P, TB], f32, tag="pst")
    t_sb = tmpp.tile([P, TB], f32, tag="tsb")

    def _emit_qkv_tt(tt):
        tsl = slice(tt * 512, (tt + 1) * 512)
        for c in range(DC):
            nc.scalar.activation(dev8[:, c, tsl], hT[:, c, tsl], AF.Identity,
                                 bias=cons[:, 20 + c:21 + c], scale=DEV_S)
        for oc in range(DC):
            pq = psum.tile([P, 512], f32, tag="ps")
            nc.tensor.matmul(pq[:], wq_sb[:, :, oc * P:(oc + 1) * P],
                             dev8[:, :, tsl], perf_mode=DR,
                             start=True, stop=True)
            nc.vector.tensor_scalar(qT[:, oc, tsl], pq[:], SC_QK8, None,
                                    op0=ALU.mult)
        for oc in range(DC):
            pk = psum.tile([P, 512], f32, tag="ps")
            nc.tensor.matmul(pk[:], wk_sb[:, :, oc * P:(oc + 1) * P],
                             dev8[:, :, tsl], perf_mode=DR,
                             start=True, stop=True)
            nc.vector.tensor_scalar(kT[:, oc, tsl], pk[:], SC_QK8, None,
                                    op0=ALU.mult)
        # v token-major [t, d] at DEV_S scale (fp8), DoubleRow over chunks
        for i in range(4):
            tb = tt * 4 + i
            pv = psum.tile([P, 512], f32, tag="ps")
            nc.tensor.matmul(pv[:, 0:D], dev8[:, :, tb * P:(tb + 1) * P],
                             wv_sb[:, :, :], perf_mode=DR,
                             start=True, stop=True)
            nc.vector.tensor_scalar(vtm[:, tb, :], pv[:, 0:D], SC_V, None,
                                    op0=ALU.mult)
            for kc in range(DC):
                nc.tensor.matmul(ps_t[:, tb:tb + 1], kT[:, kc, tb * P:(tb + 1) * P],
                                 qbb[:, kc:kc + 1], start=(kc == 0),
                                 stop=(kc == DC - 1), skip_group_check=True)
        nc.vector.tensor_scalar(t_sb[:, tt * 4:tt * 4 + 4],
                                ps_t[:, tt * 4:tt * 4 + 4], 0.5, None,
                                op0=ALU.mult)

    # normalization of window w-1 runs during window w, multiplying the AV
    # psum directly; the residual add runs on the otherwise-idle GpSimd
    def _emit_norm(accs_, rb_sb_, q0_):
        for dc in range(DC):
            tmp = tmpp.tile([P, 512], f32, tag="tmp")
            nc.vector.tensor_tensor(tmp[:], accs_[dc][:], rb_sb_[:],
                                    op=ALU.mult)
            nc.gpsimd.tensor_add(hT[:, dc, q0_:q0_ + W], hT[:, dc, q0_:q0_ + W],
                                 tmp[:])

    pend = [None]

    def _emit_attn_w(w):
        q0 = w * W
        kb_lo = 4 if w == 0 else 0
        kstart = (w - 1) * W  # global token of kb=0
        tb0 = kstart // P
        expT = expp.tile([P, 8, 512], f8, tag="exp")
        # scores (fp8 DoubleRow over feature pairs) + exp per key block.
        # The causal mask lands via a tiny id.T@mask accumulating matmul;
        # the surviving q-bias term rides in as the exp's per-partition bias.
        for kb in range(kb_lo, 8):
            kpos = kstart + kb * P
            qlo = 0 if kb < 4 else (kb - 4) * P
            qcols = W - qlo
            ps = psum.tile([P, 512], f32, tag="ps")
            nc.tensor.matmul(ps[:, 0:qcols],
                             kT[:, :, kpos:kpos + P],
                             qT[:, :, q0 + qlo:q0 + W],
                             perf_mode=DR,
                             start=True, stop=(kb < 4),
                             skip_group_check=True)
            if kb >= 4:
                nc.tensor.matmul(ps[:, 0:P], idb[:, :], maskb[:, :],
                                 start=False, stop=True, skip_group_check=True)
            nc.scalar.activation(expT[:, kb, qlo:W], ps[:, 0:qcols], AF.Exp,
                                 bias=t_sb[:, tb0 + kb:tb0 + kb + 1],
                                 scale=SC_EXP)
        if pend[0] is not None:
            _emit_norm(*pend[0])
        # AV: fp8 DoubleRow over key-block pairs, plus two solo matmuls for
        # the q-ranges where only the even block of a pair is valid.
        # start=True on the first writer of each psum column region.
        acc0 = psum.tile([P, 512], f32, tag="ps")
        acc1 = psum.tile([P, 512], f32, tag="ps")
        accs = [acc0, acc1]
        for dc in range(DC):
            dsl = slice(dc * P, (dc + 1) * P)
            if kb_lo == 0:
                nc.tensor.matmul(accs[dc][:], vtm[:, tb0:tb0 + 2, dsl],
                                 expT[:, 0:2, :], perf_mode=DR,
                                 start=True, stop=False, skip_group_check=True)
                nc.tensor.matmul(accs[dc][:], vtm[:, tb0 + 2:tb0 + 4, dsl],
                                 expT[:, 2:4, :], perf_mode=DR,
                                 start=False, stop=False, skip_group_check=True)
            nc.tensor.matmul(accs[dc][:, P:W], vtm[:, tb0 + 4:tb0 + 6, dsl],
                             expT[:, 4:6, P:W], perf_mode=DR,
                             start=(kb_lo == 4), stop=False, skip_group_check=True)
            nc.tensor.matmul(accs[dc][:, 0:P], vtm[:, tb0 + 4, dsl],
                             expT[:, 4, 0:P],
                             start=(kb_lo == 4), stop=False, skip_group_check=True)
            nc.tensor.matmul(accs[dc][:, 3 * P:W], vtm[:, tb0 + 6:tb0 + 8, dsl],
                             expT[:, 6:8, 3 * P:W], perf_mode=DR,
                             start=False, stop=False, skip_group_check=True)
            nc.tensor.matmul(accs[dc][:, 2 * P:3 * P], vtm[:, tb0 + 6, dsl],
                             expT[:, 6, 2 * P:3 * P],
                             start=False, stop=True, skip_group_check=True)
        # softmax denominators on the PE too: ones(=DEV_S) pair-matmuls give
        # the column sums broadcast across all 128 partitions
        ssum = psum.tile([P, 512], f32, tag="ps")
        if kb_lo == 0:
            nc.tensor.matmul(ssum[:], ones8[:, :, :], expT[:, 0:2, :],
                             perf_mode=DR, start=True, stop=False,
                             skip_group_check=True)
            nc.tensor.matmul(ssum[:], ones8[:, :, :], expT[:, 2:4, :],
                             perf_mode=DR, start=False, stop=False,
                             skip_group_check=True)
        nc.tensor.matmul(ssum[:, P:W], ones8[:, :, :], expT[:, 4:6, P:W],
                         perf_mode=DR, start=(kb_lo == 4), stop=False,
                         skip_group_check=True)
        nc.tensor.matmul(ssum[:, 0:P], ones8[:, 0, :], expT[:, 4, 0:P],
                         start=(kb_lo == 4), stop=False, skip_group_check=True)
        nc.tensor.matmul(ssum[:, 3 * P:W], ones8[:, :, :], expT[:, 6:8, 3 * P:W],
                         perf_mode=DR, start=False, stop=False,
                         skip_group_check=True)
        nc.tensor.matmul(ssum[:, 2 * P:3 * P], ones8[:, 0, :],
                         expT[:, 6, 2 * P:3 * P],
                         start=False, stop=True, skip_group_check=True)
        rb_sb = rbp.tile([P, 512], f32, tag="recip")
        nc.vector.reciprocal_approx_fast(out=rb_sb[:], in_=ssum[:])
        pend[0] = (accs, rb_sb, q0)

    for step in range(NW + 1):
        if step < NW:
            _emit_qkv_tt(step)
        if step >= 1:
            _emit_attn_w(step - 1)
    _emit_norm(*pend[0])

    # ---- MLP + BN, per token tile ---------------------------------------
    # dev cast of the post-attention residual (same hbar: the missing
    # vc = bv + Wv@hbar exactly cancels hbar's attention-side update).
    # ut = tanh(z/2) = 2*sigmoid(z)-1; residual+BN on the idle GpSimd.
    u8 = usb.tile([P, EC, S], f8, tag="u")

    def _emit_w2(tt):
        tsl = slice(tt * 512, (tt + 1) * 512)
        for dc in range(DC):
            pm = psum.tile([P, 512], f32, tag="ps")
            for ecp in range(EC // 2):
                nc.tensor.matmul(pm[:],
                                 w2_sb[:, ecp, :, dc * P:(dc + 1) * P],
                                 u8[:, 2 * ecp:2 * ecp + 2, tsl],
                                 perf_mode=DR,
                                 start=(ecp == 0), stop=(ecp == EC // 2 - 1))
            mlp_sb = tmpp.tile([P, 512], f32, tag="mlp")
            nc.vector.tensor_scalar(mlp_sb[:], pm[:],
                                    cons[:, 16 + dc:17 + dc], None,
                                    op0=ALU.mult)
            nc.gpsimd.tensor_scalar(hT[:, dc, tsl], hT[:, dc, tsl],
                                    cons[:, 12 + dc:13 + dc],
                                    cons[:, 14 + dc:15 + dc],
                                    op0=ALU.mult, op1=ALU.add)
            nc.gpsimd.tensor_add(hT[:, dc, tsl], hT[:, dc, tsl], mlp_sb[:])

    # W2(tt-1) is emitted after W1(tt) so the PE never waits on a tanh
    for tt in range(TT):
        tsl = slice(tt * 512, (tt + 1) * 512)
        for c in range(DC):
            nc.vector.tensor_scalar(dev8[:, c, tsl], hT[:, c, tsl],
                                    cons[:, 18 + c:19 + c], DEV_S,
                                    op0=ALU.subtract, op1=ALU.mult)
        for ec in range(EC):
            pu = psum.tile([P, 512], f32, tag="ps")
            nc.tensor.matmul(pu[:], w1_sb[:, :, ec * P:(ec + 1) * P],
                             dev8[:, :, tsl], perf_mode=DR,
                             start=True, stop=True)
            nc.scalar.activation(u8[:, ec, tsl], pu[:], AF.Tanh,
                                 bias=cons[:, 4 + ec:5 + ec], scale=SC_U)
        if tt > 0:
            _emit_w2(tt - 1)
    _emit_w2(TT - 1)


def _build(n_layers=L, unroll=False):
    nc = bacc.Bacc("TRN2", target_bir_lowering=False, debug=False)

    h0_d = nc.dram_tensor("h0T", [D, S], f32, kind="ExternalInput")
    nc.t_wq8 = nc.dram_tensor("wq8", [n_layers * P, DC * D], f8, kind="ExternalInput")
    nc.t_wk8 = nc.dram_tensor("wk8", [n_layers * P, DC * D], f8, kind="ExternalInput")
    nc.t_wv8 = nc.dram_tensor("wv8", [n_layers * P, DC * D], f8, kind="ExternalInput")
    nc.t_w18 = nc.dram_tensor("w18", [n_layers * P, DC * E], f8, kind="ExternalInput")
    nc.t_w28 = nc.dram_tensor("w28", [n_layers * P, EC * D], f8, kind="ExternalInput")
    nc.t_cons = nc.dram_tensor("cons", [n_layers * P, 22], f32, kind="ExternalInput")
    nc.t_qbb = nc.dram_tensor("qbb", [n_layers * P, DC], bf16, kind="ExternalInput")
    mask_d = nc.dram_tensor("maskT", [P, P], bf16, kind="ExternalInput")
    id_d = nc.dram_tensor("idT", [P, P], bf16, kind="ExternalInput")
    wf8_d = nc.dram_tensor("wf8", [P, DC * D], f8, kind="ExternalInput")
    bf_d = nc.dram_tensor("bfc", [P, 4], f32, kind="ExternalInput")
    out_d = nc.dram_tensor("outT", [D, S], f32, kind="ExternalOutput")

    with tile.TileContext(nc) as tc:
        with tc.tile_pool(name="persist", bufs=1) as persist, \
             tc.tile_pool(name="wpool", bufs=2) as wpool, \
             tc.tile_pool(name="psum", bufs=7, space="PSUM") as psum, \
             tc.tile_pool(name="psum2", bufs=1, space="PSUM") as psum2, \
             tc.tile_pool(name="expp", bufs=2) as expp, \
             tc.tile_pool(name="rbp", bufs=2) as rbp, \
             tc.tile_pool(name="tmpp", bufs=4) as tmpp, \
             tc.tile_pool(name="usb", bufs=1) as usb, \
             tc.tile_pool(name="outp", bufs=4) as outp:

            hT = persist.tile([P, DC, S], f32)
            dev8 = persist.tile([P, DC, S], f8)
            qT = persist.tile([P, DC, S], f8)
            kT = persist.tile([P, DC, S], f8)
            vtm = persist.tile([P, TB, D], f8)
            ones8 = persist.tile([P, DC, P], f8)
            maskb = persist.tile([P, P], bf16)
            idb = persist.tile([P, P], bf16)
            wf_sb = persist.tile([P, DC, D], f8)
            bf_sb = persist.tile([P, 4], f32)

            nc.vector.memset(ones8, DEV_S)
            nc.sync.dma_start(out=maskb, in_=mask_d[:, :])
            nc.sync.dma_start(out=idb, in_=id_d[:, :])
            for kc in range(DC):
                nc.sync.dma_start(out=hT[:, kc, :], in_=h0_d[kc * P:(kc + 1) * P, :])
            nc.sync.dma_start(out=wf_sb[:, :, :], in_=wf8_d[:, :])
            nc.sync.dma_start(out=bf_sb, in_=bf_d[:, :])

            pools = (wpool, psum, psum2, expp, rbp, tmpp, usb,
                     hT, dev8, qT, kT, vtm, ones8, maskb, idb)

            if unroll:
                for l in range(n_layers):
                    _emit_layer(nc, tc, pools, l)
            else:
                with tc.For_i(0, n_layers, 1) as lv:
                    _emit_layer(nc, tc, pools, lv)

            # final 1x1 conv + relu: dev cast with hbar_final, DoubleRow fp8
            for c in range(DC):
                for tt in range(TT):
                    tsl = slice(tt * 512, (tt + 1) * 512)
                    nc.vector.tensor_scalar(dev8[:, c, tsl], hT[:, c, tsl],
                                            bf_sb[:, 2 + c:3 + c], DEV_S,
                                            op0=ALU.subtract, op1=ALU.mult)
            for oc in range(DC):
                for tt in range(TT):
                    tsl = slice(tt * 512, (tt + 1) * 512)
                    pf = psum.tile([P, 512], f32, tag="ps")
                    nc.tensor.matmul(pf[:], wf_sb[:, :, oc * P:(oc + 1) * P],
                                     dev8[:, :, tsl], perf_mode=DR,
                                     start=True, stop=True)
                    ot = outp.tile([P, 512], f32, tag="out")
                    nc.scalar.activation(ot[:], pf[:], AF.Relu,
                                         bias=bf_sb[:, oc:oc + 1], scale=SC_F)
                    nc.sync.dma_start(out=out_d[oc * P:(oc + 1) * P, tsl], in_=ot[:])

    nc.compile()
    return nc


def _stationary_pairs(WT, scale, n_layers, kdim, mdim):
    """W: (L, mdim, kdim) -> fp8 [L*P, (kdim/P)*mdim] with layout
    [l*P+p, kc*mdim + m] = W.T[kc*128+p, m] * scale (pairs-planar)."""
    KC = kdim // P
    t = np.transpose(WT, (0, 2, 1)) * scale            # (L, kdim, mdim)
    t = t.reshape(n_layers, KC, P, mdim).transpose(0, 2, 1, 3)
    return np.ascontiguousarray(t.reshape(n_layers * P, KC * mdim)).astype(f8np)


def _prep_host(inputs, n_layers=L):
    x = np.asarray(inputs['x'])
    emb = np.asarray(inputs['emb'], np.float32)
    bn_scale = 1.0 / np.sqrt(1.0 + BN_EPS)

    Wq = np.asarray(inputs['Wq'], np.float32)[:n_layers]
    Wk = np.asarray(inputs['Wk'], np.float32)[:n_layers]
    Wv = np.asarray(inputs['Wv'], np.float32)[:n_layers]
    W1 = np.asarray(inputs['W1'], np.float32)[:n_layers]
    W2 = np.asarray(inputs['W2'], np.float32)[:n_layers]
    bq = np.asarray(inputs['bq'], np.float32)[:n_layers]
    bk = np.asarray(inputs['bk'], np.float32)[:n_layers]
    bv = np.asarray(inputs['bv'], np.float32)[:n_layers]
    b1 = np.asarray(inputs['b1'], np.float32)[:n_layers]
    b2 = np.asarray(inputs['b2'], np.float32)[:n_layers]
    gamma = np.asarray(inputs['gamma'], np.float32)[:n_layers]
    beta = np.asarray(inputs['beta'], np.float32)[:n_layers]
    Wf = np.asarray(inputs['Wf'], np.float32)
    bf = np.asarray(inputs['bf'], np.float32)

    h0 = emb[x]                                # (B, S, D) f32

    # ---- hbar recurrence on the batch-mean embedding ----
    hb = h0.mean(axis=(0, 1)).astype(np.float64)
    hbars = np.zeros((n_layers, D), np.float64)
    vcs = np.zeros((n_layers, D), np.float64)
    for l in range(n_layers):
        hbars[l] = hb
        vc = Wv[l] @ hb + bv[l]
        vcs[l] = vc
        hmid = hb + vc
        u = 1.0 / (1.0 + np.exp(-(W1[l] @ hmid + b1[l])))
        hb = gamma[l] * bn_scale * (hmid + W2[l] @ u + b2[l]) + beta[l]
    hbar_f = hb

    hbars32 = hbars.astype(np.float32)
    qbias = (bq + np.einsum('lod,ld->lo', Wq, hbars32)) * QK_INV
    kbias = bk + np.einsum('lod,ld->lo', Wk, hbars32)
    b1t = 0.5 * (b1 + np.einsum('led,ld->le', W1, (hbars + vcs).astype(np.float32)))
    b2t = b2 + vcs.astype(np.float32) + 0.5 * W2.sum(axis=2)
    A = gamma * bn_scale
    Cfull = A * b2t + beta
    A5 = A / W2_S

    wq8 = _stationary_pairs(Wq, WT_S, n_layers, D, D)
    wk8 = _stationary_pairs(Wk, WT_S, n_layers, D, D)
    wv8 = _stationary_pairs(Wv, WT_S, n_layers, D, D)
    w18 = _stationary_pairs(W1, WT_S, n_layers, D, E)
    # w2: pairs along E (ec chunks): [l*P+p, (2*ecp+j)*D + m]
    t = np.transpose(W2, (0, 2, 1)) * (0.5 * W2_S)     # (L, E, D)
    t = t.reshape(n_layers, EC, P, D).transpose(0, 2, 1, 3)
    w28 = np.ascontiguousarray(t.reshape(n_layers * P, EC * D)).astype(f8np)

    def packdc(v):       # (L, D) -> (L*P, DC) feature-chunk-major columns
        return v.reshape(n_layers, DC, P).transpose(0, 2, 1)

    qbb = np.ascontiguousarray(
        packdc(qbias).reshape(n_layers * P, DC)).astype(ml_dtypes.bfloat16)

    cons = np.zeros((n_layers, P, 22), np.float32)
    cons[:, :, 0:2] = packdc(qbias)
    cons[:, :, 2:4] = packdc(kbias)
    cons[:, :, 4:12] = b1t.reshape(n_layers, EC, P).transpose(0, 2, 1)
    cons[:, :, 12:14] = packdc(A)
    cons[:, :, 14:16] = packdc(Cfull)
    cons[:, :, 16:18] = packdc(A5)
    cons[:, :, 18:20] = packdc(hbars32)
    cons[:, :, 20:22] = packdc(-DEV_S * hbars32)
    cons = cons.reshape(n_layers * P, 22)

    r = np.arange(P)
    maskT = np.where(r[None, :] >= r[:, None], 0.0, NEG).astype(ml_dtypes.bfloat16)
    idT = np.eye(P, dtype=ml_dtypes.bfloat16)

    tf = (Wf.T * WT_S).reshape(DC, P, D).transpose(1, 0, 2)
    wf8 = np.ascontiguousarray(tf.reshape(P, DC * D)).astype(f8np)
    bfin = (bf + Wf @ hbar_f.astype(np.float32))
    bfc = np.zeros((P, 4), np.float32)
    bfc[:, 0:2] = bfin.reshape(DC, P).T
    bfc[:, 2:4] = hbar_f.astype(np.float32).reshape(DC, P).T

    shared = dict(wq8=wq8, wk8=wk8, wv8=wv8, w18=w18, w28=w28,
                  cons=cons, qbb=qbb, maskT=maskT, idT=idT, wf8=wf8, bfc=bfc)

    in_maps = []
    for b in range(B):
        m = dict(shared)
        m['h0T'] = np.ascontiguousarray(h0[b].T)   # (D, S) f32
        in_maps.append(m)
    return in_maps


def kernel(**inputs):
    global LAST_EXEC_NS, LAST_TRACE
    n_layers = int(os.environ.get('KERNEL_NLAYERS', L))
    unroll = os.environ.get('KERNEL_UNROLL', '1') == '1'
    trace = os.environ.get('KERNEL_TRACE', '0') == '1'
    if trace:
        _install_ntff_hook()

    key = (n_layers, unroll)
    if key not in _cache:
        _cache[key] = _build(n_layers=n_layers, unroll=unroll)
    nc = _cache[key]

    in_maps = _prep_host(inputs, n_layers=n_layers)
    res = run_bass_kernel_spmd(nc, in_maps, core_ids=list(range(B)), trace=trace)
    LAST_EXEC_NS = res.exec_time_ns
    LAST_TRACE = res.instructions_and_trace[1] if res.instructions_and_trace else None
    out = np.stack([res.results[b]['outT'] for b in range(B)], axis=0)
    return out


# revision 26
# speedup vs baseline: 1.1450x; 1.1450x over previous
"""Trainium2 Bass kernel for nn_GameboyNet (sparse windowed attention net).

Sharding: pure data-parallel over batch — B=8 rows, one per NeuronCore.
Each core runs the full 32-layer network on its own (S=4096, D=256)
sequence, residual stream resident in SBUF feature-major (D x S) f32.

Key trick — centered fp8: the residual stream h is ~99% a data-independent
constant hbar (accumulated biases; computed on the host by running the
layer recurrence on the batch-mean embedding). The device quantizes only
dev = (h - hbar)*64 to fp8-e4m3 and all dense projections (QKV, W1, W2,
final conv) run as fp8 DoubleRow matmuls (2 MACs/cell/cycle, K=256 pairs);
the exact hbar contributions travel through f32 bias paths folded on the
host. The MLP sigmoid is likewise centered: u = 0.5 + 0.5*tanh(z/2), the
0.5*sum(W2) part folded into the bias, so fp8 only carries tanh deviations.
Attention itself (scores, exp, AV) stays bf16.

Attention (window W=512, causal, look_backward=1) is computed block-sparse
in transposed form: scoresT[k, q] = kT.T @ qT per 128-token key block, so
the AV matmul out[d, q] lands feature-major, matching the residual layout.
Softmax skips max-subtraction (scores are small for this data regime;
validated vs reference). Denominators: exp tiles tree-added on DVE, one
ones[128x128] matmul per window broadcasts column sums to all partitions,
reciprocal_approx_fast gives 1/sum; normalization multiplies the AV psum
directly (software-pipelined one window behind). The v bias and Wv@hbar
pass through softmax exactly (weights sum to 1) and are folded into b1/b2.
"""
import os
import sys
import types

sys.path.insert(0, '/opt/trn_rl_repo')

import numpy as np
import ml_dtypes

import concourse.bass as bass
import concourse.mybir as mybir
import concourse.tile as tile
from concourse import bacc
from concourse.bass import ds
from concourse.bass_utils import run_bass_kernel_spmd

B, S, D, W, L = 8, 4096, 256, 512, 32
E = 4 * D
NW = S // W
P = 128
DC = D // P          # 2 d-chunks
EC = E // P          # 8 e-chunks
TT = S // 512        # 8 token tiles of 512
TB = S // P          # 32 token blocks of 128
BN_EPS = 1e-5
NEG = -1e9

DEV_S = 64.0         # dev = (h - hbar) * 2^6
WT_S = 16.0          # fp8 weights * 2^4
W2_S = 32.0          # w2' = 0.5*W2 * 2^5
QK_INV = 1.0 / 16.0  # 1/sqrt(D)
SC_QK8 = 1.0 / 32.0    # q/k psum -> fp8 qT/kT at 32*{q,k}_dev
SC_EXP = QK_INV / (DEV_S * WT_S)       # exp scale: scores psum * 2^-14
SC_V = 1.0 / WT_S                      # vtm8 = v_dev * DEV_S (fp8 range)
SC_U = 0.5 / (DEV_S * WT_S)            # tanh(z/2): z = psum*2^-10
SC_F = 1.0 / (DEV_S * WT_S)

f32 = mybir.dt.float32
bf16 = mybir.dt.bfloat16
f8 = mybir.dt.float8e4
f8np = ml_dtypes.float8_e4m3
AF = mybir.ActivationFunctionType
ALU = mybir.AluOpType
DR = mybir.MatmulPerfMode.DoubleRow

LAST_EXEC_NS = None
LAST_TRACE = None

_cache = {}


def _install_ntff_hook():
    """The agent image's antenv is a stub without axon_hooks; inject it so
    trace=True can capture NTFF profiles through the axon tunnel."""
    try:
        import antenv
        if 'antenv.axon_hooks' in sys.modules:
            return
        mod = types.ModuleType("antenv.axon_hooks")
        _HOOK = [None]
        mod.set_axon_ntff_profile_hook = lambda h: _HOOK.__setitem__(0, h)
        mod.get_axon_ntff_profile_hook = lambda: _HOOK[0]
        sys.modules["antenv.axon_hooks"] = mod
        antenv.axon_hooks = mod
        from trn_agent_boot.trn_boot import _ntff_profile_via_ctypes
        hook = _ntff_profile_via_ctypes('/opt/axon/libaxon_pjrt.so')
        mod.set_axon_ntff_profile_hook(hook)
    except Exception:
        pass


def _emit_layer(nc, tc, pools, loff):
    """Emit one transformer layer. loff = layer index (int or RV)."""
    (wpool, psum, psum2, expp, rbp, tmpp, usb,
     hT, dev8, qT, kT, vtm, ones8, maskb, idb) = pools

    dma = nc.sync.dma_start

    # ---- per-layer weight loads --------------------------------------
    wq_sb = wpool.tile([P, DC, D], f8, tag="wq")
    wk_sb = wpool.tile([P, DC, D], f8, tag="wk")
    wv_sb = wpool.tile([P, DC, D], f8, tag="wv")
    w1_sb = wpool.tile([P, DC, E], f8, tag="w1")
    w2_sb = wpool.tile([P, EC // 2, 2, D], f8, tag="w2")
    cons = wpool.tile([P, 22], f32, tag="cons")
    qbb = wpool.tile([P, DC], bf16, tag="qbb")

    dma(out=wq_sb[:, :, :], in_=nc.t_wq8[ds(loff * P, P), :])
    dma(out=wk_sb[:, :, :], in_=nc.t_wk8[ds(loff * P, P), :])
    dma(out=wv_sb[:, :, :], in_=nc.t_wv8[ds(loff * P, P), :])
    dma(out=w1_sb[:, :, :], in_=nc.t_w18[ds(loff * P, P), :])
    dma(out=w2_sb[:, :, :, :], in_=nc.t_w28[ds(loff * P, P), :])
    dma(out=cons, in_=nc.t_cons[ds(loff * P, P), :])
    dma(out=qbb, in_=nc.t_qbb[ds(loff * P, P), :])
    # cons cols: 0:2 qbias/16, 2:4 kbias, 4:12 b1t, 12:14 A, 14:16 Cfull,
    #            16:18 A*2^-5, 18:20 hbar, 20:22 -64*hbar

    # ---- QKV interleaved with attention: window w's matmuls run right
    # after token tile w's projections, so the QKV-phase DVE drains overlap
    # the attention-phase PE work instead of throttling it.
    # The q/k biases are dropped: only the per-key-token term (qbias . k_dev)
    # survives softmax, computed as tiny N=1 matmuls and applied as exp bias.
    ps_t = psum2.tile([P, TB], f32, tag="pst")
    t_sb = tmpp.tile([P, TB], f32, tag="tsb")

    def _emit_qkv_tt(tt):
        tsl = slice(tt * 512, (tt + 1) * 512)
        for c in range(DC):
            nc.vector.tensor_scalar(dev8[:, c, tsl], hT[:, c, tsl],
                                    cons[:, 18 + c:19 + c], DEV_S,
                                    op0=ALU.subtract, op1=ALU.mult)
        for oc in range(DC):
            pq = psum.tile([P, 512], f32, tag="ps")
            nc.tensor.matmul(pq[:], wq_sb[:, :, oc * P:(oc + 1) * P],
                             dev8[:, :, tsl], perf_mode=DR,
                             start=True, stop=True)
            nc.vector.tensor_scalar(qT[:, oc, tsl], pq[:], SC_QK8, None,
                                    op0=ALU.mult)
        for oc in range(DC):
            pk = psum.tile([P, 512], f32, tag="ps")
            nc.tensor.matmul(pk[:], wk_sb[:, :, oc * P:(oc + 1) * P],
                             dev8[:, :, tsl], perf_mode=DR,
                             start=True, stop=True)
            nc.vector.tensor_scalar(kT[:, oc, tsl], pk[:], SC_QK8, None,
                                    op0=ALU.mult)
        # v token-major [t, d] at DEV_S scale (fp8), DoubleRow over chunks
        for i in range(4):
            tb = tt * 4 + i
            pv = psum.tile([P, 512], f32, tag="ps")
            nc.tensor.matmul(pv[:, 0:D], dev8[:, :, tb * P:(tb + 1) * P],
                             wv_sb[:, :, :], perf_mode=DR,
                             start=True, stop=True)
            nc.vector.tensor_scalar(vtm[:, tb, :], pv[:, 0:D], SC_V, None,
                                    op0=ALU.mult)

    def _emit_t_tt(tt):
        for i in range(4):
            tb = tt * 4 + i
            for kc in range(DC):
                nc.tensor.matmul(ps_t[:, tb:tb + 1], kT[:, kc, tb * P:(tb + 1) * P],
                                 qbb[:, kc:kc + 1], start=(kc == 0),
                                 stop=(kc == DC - 1), skip_group_check=True)
        nc.vector.tensor_scalar(t_sb[:, tt * 4:tt * 4 + 4],
                                ps_t[:, tt * 4:tt * 4 + 4], 0.5, None,
                                op0=ALU.mult)

    # normalization of window w-1 runs during window w, multiplying the AV
    # psum directly; the residual add runs on the otherwise-idle GpSimd
    def _emit_norm(accs_, rb_sb_, q0_):
        for dc in range(DC):
            tmp = tmpp.tile([P, 512], f32, tag="tmp")
            nc.vector.tensor_tensor(tmp[:], accs_[dc][:], rb_sb_[:],
                                    op=ALU.mult)
            nc.gpsimd.tensor_add(hT[:, dc, q0_:q0_ + W], hT[:, dc, q0_:q0_ + W],
                                 tmp[:])

    pend = [None]

    def _emit_attn_w(w):
        q0 = w * W
        kb_lo = 4 if w == 0 else 0
        kstart = (w - 1) * W  # global token of kb=0
        tb0 = kstart // P
        expT = expp.tile([P, 8, 512], f8, tag="exp")
        # scores (fp8 DoubleRow over feature pairs) + exp per key block.
        # The causal mask lands via a tiny id.T@mask accumulating matmul;
        # the surviving q-bias term rides in as the exp's per-partition bias.
        for kb in range(kb_lo, 8):
            kpos = kstart + kb * P
            qlo = 0 if kb < 4 else (kb - 4) * P
            qcols = W - qlo
            ps = psum.tile([P, 512], f32, tag="ps")
            nc.tensor.matmul(ps[:, 0:qcols],
                             kT[:, :, kpos:kpos + P],
                             qT[:, :, q0 + qlo:q0 + W],
                             perf_mode=DR,
                             start=True, stop=(kb < 4),
                             skip_group_check=True)
            if kb >= 4:
                nc.tensor.matmul(ps[:, 0:P], idb[:, :], maskb[:, :],
                                 start=False, stop=True, skip_group_check=True)
            nc.scalar.activation(expT[:, kb, qlo:W], ps[:, 0:qcols], AF.Exp,
                                 bias=t_sb[:, tb0 + kb:tb0 + kb + 1],
                                 scale=SC_EXP)
        if pend[0] is not None:
            _emit_norm(*pend[0])
        # AV: fp8 DoubleRow over key-block pairs, plus two solo matmuls for
        # the q-ranges where only the even block of a pair is valid.
        # start=True on the first writer of each psum column region.
        acc0 = psum.tile([P, 512], f32, tag="ps")
        acc1 = psum.tile([P, 512], f32, tag="ps")
        accs = [acc0, acc1]
        for dc in range(DC):
            dsl = slice(dc * P, (dc + 1) * P)
            if kb_lo == 0:
                nc.tensor.matmul(accs[dc][:], vtm[:, tb0:tb0 + 2, dsl],
                                 expT[:, 0:2, :], perf_mode=DR,
                                 start=True, stop=False, skip_group_check=True)
                nc.tensor.matmul(accs[dc][:], vtm[:, tb0 + 2:tb0 + 4, dsl],
                                 expT[:, 2:4, :], perf_mode=DR,
                                 start=False, stop=False, skip_group_check=True)
            nc.tensor.matmul(accs[dc][:, P:W], vtm[:, tb0 + 4:tb0 + 6, dsl],
                             expT[:, 4:6, P:W], perf_mode=DR,
                             start=(kb_lo == 4), stop=False, skip_group_check=True)
            nc.tensor.matmul(accs[dc][:, 0:P], vtm[:, tb0 + 4, dsl],
                             expT[:, 4, 0:P],
                             start=(kb_lo == 4), stop=False, skip_group_check=True)
            nc.tensor.matmul(accs[dc][:, 3 * P:W], vtm[:, tb0 + 6:tb0 + 8, dsl],
                             expT[:, 6:8, 3 * P:W], perf_mode=DR,
                             start=False, stop=False, skip_group_check=True)
            nc.tensor.matmul(accs[dc][:, 2 * P:3 * P], vtm[:, tb0 + 6, dsl],
                             expT[:, 6, 2 * P:3 * P],
                             start=False, stop=True, skip_group_check=True)
        # softmax denominators on the PE too: ones(=DEV_S) pair-matmuls give
        # the column sums broadcast across all 128 partitions
        ssum = psum.tile([P, 512], f32, tag="ps")
        if kb_lo == 0:
            nc.tensor.matmul(ssum[:], ones8[:, :, :], expT[:, 0:2, :],
                             perf_mode=DR, start=True, stop=False,
                             skip_group_check=True)
            nc.tensor.matmul(ssum[:], ones8[:, :, :], expT[:, 2:4, :],
                             perf_mode=DR, start=False, stop=False,
                             skip_group_check=True)
        nc.tensor.matmul(ssum[:, P:W], ones8[:, :, :], expT[:, 4:6, P:W],
                         perf_mode=DR, start=(kb_lo == 4), stop=False,
                         skip_group_check=True)
        nc.tensor.matmul(ssum[:, 0:P], ones8[:, 0, :], expT[:, 4, 0:P],
                         start=(kb_lo == 4), stop=False, skip_group_check=True)
        nc.tensor.matmul(ssum[:, 3 * P:W], ones8[:, :, :], expT[:, 6:8, 3 * P:W],
                         perf_mode=DR, start=False, stop=False,
                         skip_group_check=True)
        nc.tensor.matmul(ssum[:, 2 * P:3 * P], ones8[:, 0, :],
                         expT[:, 6, 2 * P:3 * P],
                         start=False, stop=True, skip_group_check=True)
        rb_sb = rbp.tile([P, 512], f32, tag="recip")
        nc.vector.reciprocal_approx_fast(out=rb_sb[:], in_=ssum[:])
        pend[0] = (accs, rb_sb, q0)

    for step in range(NW + 1):
        if step < NW:
            _emit_qkv_tt(step)
        if step >= 1:
            _emit_t_tt(step - 1)
            _emit_attn_w(step - 1)
    _emit_norm(*pend[0])

    # ---- MLP + BN, per token tile ---------------------------------------
    # dev cast of the post-attention residual (same hbar: the missing
    # vc = bv + Wv@hbar exactly cancels hbar's attention-side update).
    # ut = tanh(z/2) = 2*sigmoid(z)-1; residual+BN on the idle GpSimd.
    u8 = usb.tile([P, EC, S], f8, tag="u")

    def _emit_w2(tt):
        tsl = slice(tt * 512, (tt + 1) * 512)
        for dc in range(DC):
            pm = psum.tile([P, 512], f32, tag="ps")
            for ecp in range(EC // 2):
                nc.tensor.matmul(pm[:],
                                 w2_sb[:, ecp, :, dc * P:(dc + 1) * P],
                                 u8[:, 2 * ecp:2 * ecp + 2, tsl],
                                 perf_mode=DR,
                                 start=(ecp == 0), stop=(ecp == EC // 2 - 1))
            mlp_sb = tmpp.tile([P, 512], f32, tag="mlp")
            nc.vector.tensor_scalar(mlp_sb[:], pm[:],
                                    cons[:, 16 + dc:17 + dc], None,
                                    op0=ALU.mult)
            nc.gpsimd.tensor_scalar(hT[:, dc, tsl], hT[:, dc, tsl],
                                    cons[:, 12 + dc:13 + dc],
                                    cons[:, 14 + dc:15 + dc],
                                    op0=ALU.mult, op1=ALU.add)
            nc.gpsimd.tensor_add(hT[:, dc, tsl], hT[:, dc, tsl], mlp_sb[:])

    # W2(tt-1) is emitted after W1(tt) so the PE never waits on a tanh
    for tt in range(TT):
        tsl = slice(tt * 512, (tt + 1) * 512)
        for c in range(DC):
            nc.vector.tensor_scalar(dev8[:, c, tsl], hT[:, c, tsl],
                                    cons[:, 18 + c:19 + c], DEV_S,
                                    op0=ALU.subtract, op1=ALU.mult)
        for ec in range(EC):
            pu = psum.tile([P, 512], f32, tag="ps")
            nc.tensor.matmul(pu[:], w1_sb[:, :, ec * P:(ec + 1) * P],
                             dev8[:, :, tsl], perf_mode=DR,
                             start=True, stop=True)
            nc.scalar.activation(u8[:, ec, tsl], pu[:], AF.Tanh,
                                 bias=cons[:, 4 + ec:5 + ec], scale=SC_U)
        if tt > 0:
            _emit_w2(tt - 1)
    _emit_w2(TT - 1)


def _build(n_layers=L, unroll=False):
    nc = bacc.Bacc("TRN2", target_bir_lowering=False, debug=False)

    h0_d = nc.dram_tensor("h0T", [D, S], f32, kind="ExternalInput")
    nc.t_wq8 = nc.dram_tensor("wq8", [n_layers * P, DC * D], f8, kind="ExternalInput")
    nc.t_wk8 = nc.dram_tensor("wk8", [n_layers * P, DC * D], f8, kind="ExternalInput")
    nc.t_wv8 = nc.dram_tensor("wv8", [n_layers * P, DC * D], f8, kind="ExternalInput")
    nc.t_w18 = nc.dram_tensor("w18", [n_layers * P, DC * E], f8, kind="ExternalInput")
    nc.t_w28 = nc.dram_tensor("w28", [n_layers * P, EC * D], f8, kind="ExternalInput")
    nc.t_cons = nc.dram_tensor("cons", [n_layers * P, 22], f32, kind="ExternalInput")
    nc.t_qbb = nc.dram_tensor("qbb", [n_layers * P, DC], bf16, kind="ExternalInput")
    mask_d = nc.dram_tensor("maskT", [P, P], bf16, kind="ExternalInput")
    id_d = nc.dram_tensor("idT", [P, P], bf16, kind="ExternalInput")
    wf8_d = nc.dram_tensor("wf8", [P, DC * D], f8, kind="ExternalInput")
    bf_d = nc.dram_tensor("bfc", [P, 4], f32, kind="ExternalInput")
    out_d = nc.dram_tensor("outT", [D, S], f32, kind="ExternalOutput")

    with tile.TileContext(nc) as tc:
        with tc.tile_pool(name="persist", bufs=1) as persist, \
             tc.tile_pool(name="wpool", bufs=2) as wpool, \
             tc.tile_pool(name="psum", bufs=7, space="PSUM") as psum, \
             tc.tile_pool(name="psum2", bufs=1, space="PSUM") as psum2, \
             tc.tile_pool(name="expp", bufs=2) as expp, \
             tc.tile_pool(name="rbp", bufs=2) as rbp, \
             tc.tile_pool(name="tmpp", bufs=4) as tmpp, \
             tc.tile_pool(name="usb", bufs=1) as usb, \
             tc.tile_pool(name="outp", bufs=4) as outp:

            hT = persist.tile([P, DC, S], f32)
            dev8 = persist.tile([P, DC, S], f8)
            qT = persist.tile([P, DC, S], f8)
            kT = persist.tile([P, DC, S], f8)
            vtm = persist.tile([P, TB, D], f8)
            ones8 = persist.tile([P, DC, P], f8)
            maskb = persist.tile([P, P], bf16)
            idb = persist.tile([P, P], bf16)
            wf_sb = persist.tile([P, DC, D], f8)
            bf_sb = persist.tile([P, 4], f32)

            nc.vector.memset(ones8, DEV_S)
            nc.sync.dma_start(out=maskb, in_=mask_d[:, :])
            nc.sync.dma_start(out=idb, in_=id_d[:, :])
            for kc in range(DC):
                nc.sync.dma_start(out=hT[:, kc, :], in_=h0_d[kc * P:(kc + 1) * P, :])
            nc.sync.dma_start(out=wf_sb[:, :, :], in_=wf8_d[:, :])
            nc.sync.dma_start(out=bf_sb, in_=bf_d[:, :])

            pools = (wpool, psum, psum2, expp, rbp, tmpp, usb,
                     hT, dev8, qT, kT, vtm, ones8, maskb, idb)

            if unroll:
                for l in range(n_layers):
                    _emit_layer(nc, tc, pools, l)
            else:
                with tc.For_i(0, n_layers, 1) as lv:
                    _emit_layer(nc, tc, pools, lv)

            # final 1x1 conv + relu: dev cast with hbar_final, DoubleRow fp8
            for c in range(DC):
                for tt in range(TT):
                    tsl = slice(tt * 512, (tt + 1) * 512)
                    nc.vector.tensor_scalar(dev8[:, c, tsl], hT[:, c, tsl],
                                            bf_sb[:, 2 + c:3 + c], DEV_S,
                                            op0=ALU.subtract, op1=ALU.mult)
            for oc in range(DC):
                for tt in range(TT):
                    tsl = slice(tt * 512, (tt + 1) * 512)
                    pf = psum.tile([P, 512], f32, tag="ps")
                    nc.tensor.matmul(pf[:], wf_sb[:, :, oc * P:(oc + 1) * P],
                                     dev8[:, :, tsl], perf_mode=DR,
                                     start=True, stop=True)
                    ot = outp.tile([P, 512], f32, tag="out")
                    nc.scalar.activation(ot[:], pf[:], AF.Relu,
                                         bias=bf_sb[:, oc:oc + 1], scale=SC_F)
                    nc.sync.dma_start(out=out_d[oc * P:(oc + 1) * P, tsl], in_=ot[:])

    nc.compile()
    return nc


def _stationary_pairs(WT, scale, n_layers, kdim, mdim):
    """W: (L, mdim, kdim) -> fp8 [L*P, (kdim/P)*mdim] with layout
    [l*P+p, kc*mdim + m] = W.T[kc*128+p, m] * scale (pairs-planar)."""
    KC = kdim // P
    t = np.transpose(WT, (0, 2, 1)) * scale            # (L, kdim, mdim)
    t = t.reshape(n_layers, KC, P, mdim).transpose(0, 2, 1, 3)
    return np.ascontiguousarray(t.reshape(n_layers * P, KC * mdim)).astype(f8np)


def _prep_host(inputs, n_layers=L):
    x = np.asarray(inputs['x'])
    emb = np.asarray(inputs['emb'], np.float32)
    bn_scale = 1.0 / np.sqrt(1.0 + BN_EPS)

    Wq = np.asarray(inputs['Wq'], np.float32)[:n_layers]
    Wk = np.asarray(inputs['Wk'], np.float32)[:n_layers]
    Wv = np.asarray(inputs['Wv'], np.float32)[:n_layers]
    W1 = np.asarray(inputs['W1'], np.float32)[:n_layers]
    W2 = np.asarray(inputs['W2'], np.float32)[:n_layers]
    bq = np.asarray(inputs['bq'], np.float32)[:n_layers]
    bk = np.asarray(inputs['bk'], np.float32)[:n_layers]
    bv = np.asarray(inputs['bv'], np.float32)[:n_layers]
    b1 = np.asarray(inputs['b1'], np.float32)[:n_layers]
    b2 = np.asarray(inputs['b2'], np.float32)[:n_layers]
    gamma = np.asarray(inputs['gamma'], np.float32)[:n_layers]
    beta = np.asarray(inputs['beta'], np.float32)[:n_layers]
    Wf = np.asarray(inputs['Wf'], np.float32)
    bf = np.asarray(inputs['bf'], np.float32)

    h0 = emb[x]                                # (B, S, D) f32

    # ---- hbar recurrence on the batch-mean embedding ----
    hb = h0.mean(axis=(0, 1)).astype(np.float64)
    hbars = np.zeros((n_layers, D), np.float64)
    vcs = np.zeros((n_layers, D), np.float64)
    for l in range(n_layers):
        hbars[l] = hb
        vc = Wv[l] @ hb + bv[l]
        vcs[l] = vc
        hmid = hb + vc
        u = 1.0 / (1.0 + np.exp(-(W1[l] @ hmid + b1[l])))
        hb = gamma[l] * bn_scale * (hmid + W2[l] @ u + b2[l]) + beta[l]
    hbar_f = hb

    hbars32 = hbars.astype(np.float32)
    qbias = (bq + np.einsum('lod,ld->lo', Wq, hbars32)) * QK_INV
    kbias = bk + np.einsum('lod,ld->lo', Wk, hbars32)
    b1t = 0.5 * (b1 + np.einsum('led,ld->le', W1, (hbars + vcs).astype(np.float32)))
    b2t = b2 + vcs.astype(np.float32) + 0.5 * W2.sum(axis=2)
    A = gamma * bn_scale
    Cfull = A * b2t + beta
    A5 = A / W2_S

    wq8 = _stationary_pairs(Wq, WT_S, n_layers, D, D)
    wk8 = _stationary_pairs(Wk, WT_S, n_layers, D, D)
    wv8 = _stationary_pairs(Wv, WT_S, n_layers, D, D)
    w18 = _stationary_pairs(W1, WT_S, n_layers, D, E)
    # w2: pairs along E (ec chunks): [l*P+p, (2*ecp+j)*D + m]
    t = np.transpose(W2, (0, 2, 1)) * (0.5 * W2_S)     # (L, E, D)
    t = t.reshape(n_layers, EC, P, D).transpose(0, 2, 1, 3)
    w28 = np.ascontiguousarray(t.reshape(n_layers * P, EC * D)).astype(f8np)

    def packdc(v):       # (L, D) -> (L*P, DC) feature-chunk-major columns
        return v.reshape(n_layers, DC, P).transpose(0, 2, 1)

    qbb = np.ascontiguousarray(
        packdc(qbias).reshape(n_layers * P, DC)).astype(ml_dtypes.bfloat16)

    cons = np.zeros((n_layers, P, 22), np.float32)
    cons[:, :, 0:2] = packdc(qbias)
    cons[:, :, 2:4] = packdc(kbias)
    cons[:, :, 4:12] = b1t.reshape(n_layers, EC, P).transpose(0, 2, 1)
    cons[:, :, 12:14] = packdc(A)
    cons[:, :, 14:16] = packdc(Cfull)
    cons[:, :, 16:18] = packdc(A5)
    cons[:, :, 18:20] = packdc(hbars32)
    cons[:, :, 20:22] = packdc(-DEV_S * hbars32)
    cons = cons.reshape(n_layers * P, 22)

    r = np.arange(P)
    maskT = np.where(r[None, :] >= r[:, None], 0.0, NEG).astype(ml_dtypes.bfloat16)
    idT = np.eye(P, dtype=ml_dtypes.bfloat16)

    tf = (Wf.T * WT_S).reshape(DC, P, D).transpose(1, 0, 2)
    wf8 = np.ascontiguousarray(tf.reshape(P, DC * D)).astype(f8np)
    bfin = (bf + Wf @ hbar_f.astype(np.float32))
    bfc = np.zeros((P, 4), np.float32)
    bfc[:, 0:2] = bfin.reshape(DC, P).T
    bfc[:, 2:4] = hbar_f.astype(np.float32).reshape(DC, P).T

    shared = dict(wq8=wq8, wk8=wk8, wv8=wv8, w18=w18, w28=w28,
                  cons=cons, qbb=qbb, maskT=maskT, idT=idT, wf8=wf8, bfc=bfc)

    in_maps = []
    for b in range(B):
        m = dict(shared)
        m['h0T'] = np.ascontiguousarray(h0[b].T)   # (D, S) f32
        in_maps.append(m)
    return in_maps


def kernel(**inputs):
    global LAST_EXEC_NS, LAST_TRACE
    n_layers = int(os.environ.get('KERNEL_NLAYERS', L))
    unroll = os.environ.get('KERNEL_UNROLL', '1') == '1'
    trace = os.environ.get('KERNEL_TRACE', '0') == '1'
    if trace:
        _install_ntff_hook()

    key = (n_layers, unroll)
    if key not in _cache:
        _cache[key] = _build(n_layers=n_layers, unroll=unroll)
    nc = _cache[key]

    in_maps = _prep_host(inputs, n_layers=n_layers)
    res = run_bass_kernel_spmd(nc, in_maps, core_ids=list(range(B)), trace=trace)
    LAST_EXEC_NS = res.exec_time_ns
    LAST_TRACE = res.instructions_and_trace[1] if res.instructions_and_trace else None
    out = np.stack([res.results[b]['outT'] for b in range(B)], axis=0)
    return out
